# revision 9
# baseline (speedup 1.0000x reference)
"""Trainium2 Bass kernel for nn_BaselineTargetHead (per-sample dynamic MLP).

Strategy: data-parallel over 8 NeuronCores, 8 samples per core.
Per sample the chain is 5 per-sample linear layers over 64 spatial positions:
  [1024,2048] @ [2048,64] -> sigmoid -> ... -> [1,128] @ [128,64] + b

fc1-fc4 weights (99.9% of bytes) and the input x ship as fp8 e3m4 (4
mantissa bits). Host pre-scales weights by 64 (x by 2) to center N(0,0.02)
data in e3m4's normal range; the inverse scale folds into the ScalarE
activation's `scale`. fc5 weights stay fp16: the output is a 128-term dot
product with no downstream averaging, so fc5 quantization dominates the
error budget (quantizing w5 alone costs 1.4e-2 rel err; w1-w4 cost ~1e-3).

With fp8 the kernel is Tensor-engine bound (~47 ns per LDWEIGHTS+MATMUL
pair, 171 pairs/sample), so the DMA plan only has to keep the PE fed:
  - everything lives in SBUF simultaneously (~186 KB/partition), so all
    weight DMAs are issued upfront and split across BOTH HWDGE queues
    (SP: fc1 k=0..11 per sample; ACT: fc1 k=12..15 + fc2-4 per sample)
    to halve the stream time. No tile rotation, no flow-control stalls.
  - sample 0's fc1 arrives in 4-KB-column chunks so the PE starts ~5 us
    earlier; x rides first on the ACT queue for the same reason.
  - matmul: lhsT = W^T tile [128(Cin), 128(Cout)] fp8 (FWL halves the
    weight-load time), rhs = activation tile [128(Cin), 64(spatial)] fp16,
    accumulated over Cin tiles in PSUM fp32. ScalarE applies
    scale+bias+sigmoid fused, writing fp16 tiles that feed the next layer
    without any transposition.
"""

import numpy as np
import ml_dtypes

import concourse.bass as bass
import concourse.mybir as mybir
import concourse.tile as tile
from concourse.bass_utils import run_bass_kernel_spmd

N_CORES = 8
B = 64
S_PER_CORE = B // N_CORES  # 8 samples per core
HW = 64  # 8x8 spatial positions
LAYERS = [(2048, 1024), (1024, 512), (512, 256), (256, 128)]  # (Cin, Cout) of fc1..fc4
W_SCALE_FP8 = 64.0  # host multiplies fp8 weights by this; kernel divides back
X_SCALE_FP8 = 2.0  # same for the input x image
K_LO = 12  # fc1 k-tiles 0..11 on the SP queue, 12..15 on the ACT queue
LO_COLS = K_LO * LAYERS[0][1]  # 12288
HI_COLS = (16 - K_LO) * LAYERS[0][1] + sum(
    (ci // 128) * co for ci, co in LAYERS[1:]
)  # 4096 + 5376 = 9472
X_COLS = (2048 // 128) * HW  # 1024
W5_COLS = 32  # w5 zero-padded to 32 cols for a legal M=32 matmul
# bias image columns per sample: fc1 m0..7 | fc2 m0..3 | fc3 m0..1 | fc4 m0 | fc5
BIAS_COL0 = [0, 8, 12, 14]
BIAS_COLS = 16
# per-layer PSUM scale to undo the host-side fp8 pre-scaling
ACT_SCALE = [
    1.0 / (W_SCALE_FP8 * X_SCALE_FP8),
    1.0 / W_SCALE_FP8,
    1.0 / W_SCALE_FP8,
    1.0 / W_SCALE_FP8,
]
# per-layer base column of each layer's weights within a sample's HI image
HI_OFF = [4096, 4096 + 4096, 4096 + 4096 + 1024]  # fc2, fc3, fc4


def _split_ctrl_multiwaits(nc):
    """walrus in this env rejects >1 sync-wait per instruction. Move extra
    waits onto NOPs placed immediately before, on the same engine — engines
    execute in order, so this is semantically identical."""
    n_fixed = 0
    for bb in nc.main_func.blocks:
        insts = bb.instructions
        i = 0
        while i < len(insts):
            ins = insts[i]
            si = ins.sync_info
            if si is not None and si.on_wait and len(si.on_wait) > 1:
                waits = list(si.on_wait)
                new_nops = []
                for j, w in enumerate(waits[1:]):
                    nop = mybir.InstNoOp(name=f"{ins.name}-splitw-{j}", ins=[], outs=[])
                    nop.engine = ins.engine
                    nop.sync_info = mybir.SyncInfo(on_update=[], on_wait=[w])
                    new_nops.append(nop)
                si.on_wait = [waits[0]]
                insts[i:i] = new_nops
                i += len(new_nops)
                n_fixed += 1
            i += 1
    return n_fixed


def _build_nc():
    f8 = mybir.dt.float8e3
    f16 = mybir.dt.float16
    f32 = mybir.dt.float32
    nc = bass.Bass()
    wlo_d = nc.dram_tensor("wlo", [S_PER_CORE, 128, LO_COLS], f8, kind="ExternalInput")
    whi_d = nc.dram_tensor("whi", [S_PER_CORE, 128, HI_COLS], f8, kind="ExternalInput")
    ximg_d = nc.dram_tensor("ximg", [128, S_PER_CORE * X_COLS], f8, kind="ExternalInput")
    w5img_d = nc.dram_tensor("w5img", [128, S_PER_CORE * W5_COLS], f16, kind="ExternalInput")
    bias_d = nc.dram_tensor("bias", [128, S_PER_CORE * BIAS_COLS], f32, kind="ExternalInput")
    out_d = nc.dram_tensor("out", [1, S_PER_CORE * HW], f32, kind="ExternalOutput")

    sig = mybir.ActivationFunctionType.Sigmoid
    ident = mybir.ActivationFunctionType.Identity

    with tile.TileContext(nc) as tc:
        with (
            tc.tile_pool(name="wpool", bufs=1) as wpool,
            tc.tile_pool(name="qpool", bufs=2) as qpool,
            tc.tile_pool(name="psum", bufs=6, space="PSUM") as psum_pool,
        ):
            # ---- all DMAs issued upfront; everything fits in SBUF ----
            # ACT queue: x first (gates sample 0's fc1), then bias/w5, then
            # the per-sample HI weight images.
            x_sb = wpool.tile([128, S_PER_CORE * X_COLS], f8)
            nc.scalar.dma_start(x_sb[:, 0:X_COLS], ximg_d[:, 0:X_COLS])
            nc.scalar.dma_start(
                x_sb[:, X_COLS:], ximg_d[:, X_COLS:]
            )
            bias_sb = wpool.tile([128, S_PER_CORE * BIAS_COLS], f32)
            nc.scalar.dma_start(bias_sb[:], bias_d[:])
            w5_sb = wpool.tile([128, S_PER_CORE * W5_COLS], f16)
            nc.scalar.dma_start(w5_sb[:], w5img_d[:])
            whi_sb = []
            for s in range(S_PER_CORE):
                t = wpool.tile([128, HI_COLS], f8, name=f"whi{s}")
                nc.scalar.dma_start(t[:], whi_d[s, :, :])
                whi_sb.append(t)
            # SP queue: per-sample fc1 k=0..11; sample 0 in 3 chunks so the
            # PE starts as soon as the first third lands.
            wlo_sb = []
            for s in range(S_PER_CORE):
                t = wpool.tile([128, LO_COLS], f8, name=f"wlo{s}")
                if s == 0:
                    for j in range(3):
                        nc.sync.dma_start(
                            t[:, j * 4096 : (j + 1) * 4096],
                            wlo_d[s, :, j * 4096 : (j + 1) * 4096],
                        )
                else:
                    nc.sync.dma_start(t[:], wlo_d[s, :, :])
                wlo_sb.append(t)
            # all samples' outputs land in partition 0 of one tile
            # (sample s -> columns s*HW..(s+1)*HW) so one DMA ships them all
            ot_all = wpool.tile([128, S_PER_CORE * HW], f32)

            # ---- compute ----
            for s in range(S_PER_CORE):
                q_prev = x_sb[:, s * X_COLS : (s + 1) * X_COLS]
                for li, (cin, cout) in enumerate(LAYERS):
                    kt, mt = cin // 128, cout // 128
                    qn = qpool.tile([128, mt * HW], f16, tag=f"q{li}")
                    for m in range(mt):
                        ps = psum_pool.tile([128, HW], f32, tag="ps")
                        for k in range(kt):
                            if li == 0 and k < K_LO:
                                wt, wcol = wlo_sb[s], k * cout + m * 128
                            elif li == 0:
                                wt, wcol = whi_sb[s], (k - K_LO) * cout + m * 128
                            else:
                                wt, wcol = whi_sb[s], HI_OFF[li - 1] + k * cout + m * 128
                            lhsT = wt[:, wcol : wcol + 128]
                            rhs = q_prev[:, k * HW : (k + 1) * HW]
                            nc.tensor.matmul(
                                ps[:], lhsT, rhs, start=(k == 0), stop=(k == kt - 1)
                            )
                        bcol = s * BIAS_COLS + BIAS_COL0[li] + m
                        nc.scalar.activation(
                            qn[:, m * HW : (m + 1) * HW],
                            ps[:],
                            sig,
                            bias=bias_sb[:, bcol : bcol + 1],
                            scale=ACT_SCALE[li],
                        )
                    q_prev = qn[:]

                ps5 = psum_pool.tile([128, HW], f32, tag="ps", name=f"ps5_{s}")
                w5t = w5_sb[:, s * W5_COLS : (s + 1) * W5_COLS]
                nc.tensor.matmul(
                    ps5[0:32, :], w5t, q_prev[:, 0:HW], start=True, stop=True
                )
                b5col = s * BIAS_COLS + 15
                nc.scalar.activation(
                    ot_all[0:1, s * HW : (s + 1) * HW],
                    ps5[0:1, :],
                    ident,
                    bias=bias_sb[0:1, b5col : b5col + 1],
                    scale=1.0,
                )
            nc.scalar.dma_start(out_d[:, :], ot_all[0:1, :])

    _split_ctrl_multiwaits(nc)
    return nc


_NC_CACHE = None


def _get_nc():
    global _NC_CACHE
    if _NC_CACHE is None:
        _NC_CACHE = _build_nc()
    return _NC_CACHE


def _to_e3m4(a, scale):
    return np.clip(a * scale, -14.0, 14.0).astype(ml_dtypes.float8_e3m4)


def _prep_core(inputs, c):
    """Build the per-core input map (numpy only, host-side layout prep)."""
    sl = slice(c * S_PER_CORE, (c + 1) * S_PER_CORE)

    def wimg(li):
        cin, cout = LAYERS[li]
        w = inputs[f"target_fc{li + 1}w"][sl, :, :, 0, 0]  # [S, Cout, Cin]
        # -> [S, 128, (Cin/128)*Cout] with img[s, p, k*Cout+co] = w[s, co, k*128+p]
        wt = w.transpose(0, 2, 1).reshape(S_PER_CORE, cin // 128, 128, cout)
        return wt.transpose(0, 2, 1, 3).reshape(S_PER_CORE, 128, -1)

    w1 = wimg(0)  # [S, 128, 16384]
    wlo = np.ascontiguousarray(_to_e3m4(w1[:, :, :LO_COLS], W_SCALE_FP8))
    whi = np.ascontiguousarray(
        _to_e3m4(
            np.concatenate([w1[:, :, LO_COLS:]] + [wimg(li) for li in (1, 2, 3)], axis=2),
            W_SCALE_FP8,
        )
    )

    x = inputs["target_in_vec"][sl].reshape(S_PER_CORE, 2048 // 128, 128, HW)
    ximg = x.transpose(2, 0, 1, 3).reshape(128, S_PER_CORE * X_COLS)
    ximg = np.ascontiguousarray(_to_e3m4(ximg, X_SCALE_FP8))

    w5 = inputs["target_fc5w"][sl, 0, :, 0, 0].astype(np.float16)  # [S, 128]
    w5img = np.zeros((128, S_PER_CORE, W5_COLS), np.float16)
    w5img[:, :, 0] = w5.T
    w5img = np.ascontiguousarray(w5img.reshape(128, -1))

    bias = np.zeros((S_PER_CORE, 128, BIAS_COLS), np.float32)
    for li, (cin, cout) in enumerate(LAYERS):
        b = inputs[f"target_fc{li + 1}b"][sl]  # [S, Cout]
        bias[:, :, BIAS_COL0[li] : BIAS_COL0[li] + cout // 128] = b.reshape(
            S_PER_CORE, cout // 128, 128
        ).transpose(0, 2, 1)
    bias[:, 0, 15] = inputs["target_fc5b"][sl, 0]
    bias = np.ascontiguousarray(bias.transpose(1, 0, 2).reshape(128, -1))

    return {"wlo": wlo, "whi": whi, "ximg": ximg, "w5img": w5img, "bias": bias}


def kernel(**inputs):
    inputs = {k: np.asarray(v) for k, v in inputs.items()}
    nc = _get_nc()
    in_maps = [_prep_core(inputs, c) for c in range(N_CORES)]
    res = run_bass_kernel_spmd(nc, in_maps, list(range(N_CORES)))
    out = np.concatenate([np.asarray(res.results[c]["out"]) for c in range(N_CORES)], axis=0)
    return out.reshape(B, 8, 8).astype(np.float32)


# revision 18
# speedup vs baseline: 1.1761x; 1.1761x over previous
"""Trainium2 Bass kernel for nn_BaselineTargetHead (per-sample dynamic MLP).

Strategy: data-parallel over 8 NeuronCores, 8 samples per core.
Per sample the chain is 5 per-sample linear layers over 64 spatial positions:
  [1024,2048] @ [2048,64] -> sigmoid -> ... -> [1,128] @ [128,64] + b

fc1-fc4 weights (99.9% of bytes) and the input x ship as fp8 e3m4 (4
mantissa bits). Host pre-scales weights by 64 (x by 2) to center N(0,0.02)
data in e3m4's normal range; the inverse scale folds into the ScalarE
activation's `scale`. fc5 weights stay fp16: the output is a 128-term dot
product with no downstream averaging, so fc5 quantization dominates the
error budget (quantizing w5 alone costs 1.4e-2 rel err; w1-w4 cost ~1e-3).

With fp8 the kernel sits at the ridge: Tensor ~65 us busy (47 ns per
LDWEIGHTS+MATMUL pair, 171 pairs/sample) vs DMA ~67 us (the two HWDGE
queues share ~370 B/ns of fabric, so a single weight stream is optimal):
  - everything lives in SBUF simultaneously (~186 KB/partition), so all
    DMAs are issued upfront with no tile rotation or flow-control stalls.
    The SP queue carries only the weight slabs, sample-major, fc1 split
    into 2 chunks (4 for sample 0) so compute tracks the stream closely.
  - sample 0's x slice is the very first SP transfer (it gates the first
    matmul); the rest of x, bias and w5 ride the ACT queue which drains
    early, leaving it free for the per-sample output DMAs.
  - matmul: lhsT = W^T tile [128(Cin), 128(Cout)] fp8 (FWL halves the
    weight-load time), rhs = activation tile [128(Cin), 64(spatial)] fp16,
    accumulated over Cin tiles in PSUM fp32. ScalarE applies
    scale+bias+sigmoid fused, writing fp16 tiles that feed the next layer
    without any transposition.
"""

import numpy as np
import ml_dtypes

import concourse.bass as bass
import concourse.mybir as mybir
import concourse.tile as tile
from concourse.bass_utils import run_bass_kernel_spmd

N_CORES = 8
B = 64
S_PER_CORE = B // N_CORES  # 8 samples per core
HW = 64  # 8x8 spatial positions
LAYERS = [(2048, 1024), (1024, 512), (512, 256), (256, 128)]  # (Cin, Cout) of fc1..fc4
W_SCALE_FP8 = 64.0  # host multiplies fp8 weights by this; kernel divides back
X_SCALE_FP8 = 2.0  # same for the input x image
A_COLS = (LAYERS[0][0] // 128) * LAYERS[0][1]  # 16384 (fc1)
B_COLS = sum((ci // 128) * co for ci, co in LAYERS[1:])  # 5376 (fc2-4)
X_COLS = (2048 // 128) * HW  # 1024
W5_COLS = 32  # w5 zero-padded to 32 cols for a legal M=32 matmul
# bias image columns per sample: fc1 m0..7 | fc2 m0..3 | fc3 m0..1 | fc4 m0 | fc5
BIAS_COL0 = [0, 8, 12, 14]
BIAS_COLS = 16
# per-layer PSUM scale to undo the host-side fp8 pre-scaling
ACT_SCALE = [
    1.0 / (W_SCALE_FP8 * X_SCALE_FP8),
    1.0 / W_SCALE_FP8,
    1.0 / W_SCALE_FP8,
    1.0 / W_SCALE_FP8,
]
# per-layer base column of each layer's weights within a sample's B image
B_OFF = [0, 4096, 4096 + 1024]  # fc2, fc3, fc4


def _split_ctrl_multiwaits(nc):
    """walrus in this env rejects >1 sync-wait per instruction. Move extra
    waits onto NOPs placed immediately before, on the same engine — engines
    execute in order, so this is semantically identical."""
    n_fixed = 0
    for bb in nc.main_func.blocks:
        insts = bb.instructions
        i = 0
        while i < len(insts):
            ins = insts[i]
            si = ins.sync_info
            if si is not None and si.on_wait and len(si.on_wait) > 1:
                waits = list(si.on_wait)
                new_nops = []
                for j, w in enumerate(waits[1:]):
                    nop = mybir.InstNoOp(name=f"{ins.name}-splitw-{j}", ins=[], outs=[])
                    nop.engine = ins.engine
                    nop.sync_info = mybir.SyncInfo(on_update=[], on_wait=[w])
                    new_nops.append(nop)
                si.on_wait = [waits[0]]
                insts[i:i] = new_nops
                i += len(new_nops)
                n_fixed += 1
            i += 1
    return n_fixed


def _build_nc():
    f8 = mybir.dt.float8e3
    f16 = mybir.dt.float16
    f32 = mybir.dt.float32
    nc = bass.Bass()
    wslab_d = nc.dram_tensor(
        "wslab", [S_PER_CORE, 128, A_COLS + B_COLS], f8, kind="ExternalInput"
    )
    ximg_d = nc.dram_tensor("ximg", [128, S_PER_CORE * X_COLS], f8, kind="ExternalInput")
    w5img_d = nc.dram_tensor("w5img", [128, S_PER_CORE * W5_COLS], f16, kind="ExternalInput")
    bias_d = nc.dram_tensor("bias", [128, S_PER_CORE * BIAS_COLS], f32, kind="ExternalInput")
    out_d = nc.dram_tensor("out", [1, S_PER_CORE * HW], f32, kind="ExternalOutput")

    sig = mybir.ActivationFunctionType.Sigmoid
    ident = mybir.ActivationFunctionType.Identity

    with tile.TileContext(nc) as tc:
        with (
            tc.tile_pool(name="wpool", bufs=1) as wpool,
            tc.tile_pool(name="qpool", bufs=2) as qpool,
            tc.tile_pool(name="psum", bufs=6, space="PSUM") as psum_pool,
        ):
            # ---- all DMAs issued upfront; everything fits in SBUF ----
            # SP queue: sample 0's x slice first (it gates the first matmul),
            # then the weight slabs, sample-major. fc1 in 2 chunks (4 for
            # sample 0) so compute tracks the stream closely.
            x_sb = wpool.tile([128, S_PER_CORE * X_COLS], f8)
            nc.sync.dma_start(x_sb[:, 0:X_COLS], ximg_d[:, 0:X_COLS])
            wa_sb, wb_sb = [], []
            for s in range(S_PER_CORE):
                ta = wpool.tile([128, A_COLS], f8, name=f"wa{s}")
                nchunk = 4 if s == 0 else 2
                step = A_COLS // nchunk
                for j in range(nchunk):
                    nc.sync.dma_start(
                        ta[:, j * step : (j + 1) * step],
                        wslab_d[s, :, j * step : (j + 1) * step],
                    )
                wa_sb.append(ta)
                tb = wpool.tile([128, B_COLS], f8, name=f"wb{s}")
                nc.sync.dma_start(tb[:], wslab_d[s, :, A_COLS : A_COLS + B_COLS])
                wb_sb.append(tb)
            # ACT queue: rest of x, bias, w5 — all needed only after ~15 us,
            # and this queue drains early so per-sample output DMAs are
            # never stuck behind weight traffic.
            nc.scalar.dma_start(x_sb[:, X_COLS:], ximg_d[:, X_COLS:])
            bias_sb = wpool.tile([128, S_PER_CORE * BIAS_COLS], f32)
            nc.scalar.dma_start(bias_sb[:], bias_d[:])
            w5_sb = wpool.tile([128, S_PER_CORE * W5_COLS], f16)
            nc.scalar.dma_start(w5_sb[:], w5img_d[:])
            # all samples' outputs land in partition 0 of one tile
            # (sample s -> columns s*HW..(s+1)*HW) so one DMA ships them all
            ot_all = wpool.tile([128, S_PER_CORE * HW], f32)

            # ---- compute ----
            for s in range(S_PER_CORE):
                q_prev = x_sb[:, s * X_COLS : (s + 1) * X_COLS]
                for li, (cin, cout) in enumerate(LAYERS):
                    kt, mt = cin // 128, cout // 128
                    qn = qpool.tile([128, mt * HW], f16, tag=f"q{li}")
                    for m in range(mt):
                        ps = psum_pool.tile([128, HW], f32, tag="ps")
                        for k in range(kt):
                            if li == 0:
                                wt, wcol = wa_sb[s], k * cout + m * 128
                            else:
                                wt, wcol = wb_sb[s], B_OFF[li - 1] + k * cout + m * 128
                            lhsT = wt[:, wcol : wcol + 128]
                            rhs = q_prev[:, k * HW : (k + 1) * HW]
                            nc.tensor.matmul(
                                ps[:], lhsT, rhs, start=(k == 0), stop=(k == kt - 1)
                            )
                        bcol = s * BIAS_COLS + BIAS_COL0[li] + m
                        nc.scalar.activation(
                            qn[:, m * HW : (m + 1) * HW],
                            ps[:],
                            sig,
                            bias=bias_sb[:, bcol : bcol + 1],
                            scale=ACT_SCALE[li],
                        )
                    q_prev = qn[:]

                ps5 = psum_pool.tile([128, HW], f32, tag="ps", name=f"ps5_{s}")
                w5t = w5_sb[:, s * W5_COLS : (s + 1) * W5_COLS]
                nc.tensor.matmul(
                    ps5[0:32, :], w5t, q_prev[:, 0:HW], start=True, stop=True
                )
                b5col = s * BIAS_COLS + 15
                nc.scalar.activation(
                    ot_all[0:1, s * HW : (s + 1) * HW],
                    ps5[0:1, :],
                    ident,
                    bias=bias_sb[0:1, b5col : b5col + 1],
                    scale=1.0,
                )
                nc.scalar.dma_start(
                    out_d[0:1, s * HW : (s + 1) * HW],
                    ot_all[0:1, s * HW : (s + 1) * HW],
                )

    _split_ctrl_multiwaits(nc)
    return nc


_NC_CACHE = None


def _get_nc():
    global _NC_CACHE
    if _NC_CACHE is None:
        _NC_CACHE = _build_nc()
    return _NC_CACHE


def _to_e3m4(a, scale):
    return np.clip(a * scale, -14.0, 14.0).astype(ml_dtypes.float8_e3m4)


def _prep_core(inputs, c):
    """Build the per-core input map (numpy only, host-side layout prep)."""
    sl = slice(c * S_PER_CORE, (c + 1) * S_PER_CORE)

    def wimg(li):
        cin, cout = LAYERS[li]
        w = inputs[f"target_fc{li + 1}w"][sl, :, :, 0, 0]  # [S, Cout, Cin]
        # -> [S, 128, (Cin/128)*Cout] with img[s, p, k*Cout+co] = w[s, co, k*128+p]
        wt = w.transpose(0, 2, 1).reshape(S_PER_CORE, cin // 128, 128, cout)
        return wt.transpose(0, 2, 1, 3).reshape(S_PER_CORE, 128, -1)

    wslab = np.ascontiguousarray(
        _to_e3m4(np.concatenate([wimg(li) for li in range(4)], axis=2), W_SCALE_FP8)
    )

    x = inputs["target_in_vec"][sl].reshape(S_PER_CORE, 2048 // 128, 128, HW)
    ximg = x.transpose(2, 0, 1, 3).reshape(128, S_PER_CORE * X_COLS)
    ximg = np.ascontiguousarray(_to_e3m4(ximg, X_SCALE_FP8))

    w5 = inputs["target_fc5w"][sl, 0, :, 0, 0].astype(np.float16)  # [S, 128]
    w5img = np.zeros((128, S_PER_CORE, W5_COLS), np.float16)
    w5img[:, :, 0] = w5.T
    w5img = np.ascontiguousarray(w5img.reshape(128, -1))

    bias = np.zeros((S_PER_CORE, 128, BIAS_COLS), np.float32)
    for li, (cin, cout) in enumerate(LAYERS):
        b = inputs[f"target_fc{li + 1}b"][sl]  # [S, Cout]
        bias[:, :, BIAS_COL0[li] : BIAS_COL0[li] + cout // 128] = b.reshape(
            S_PER_CORE, cout // 128, 128
        ).transpose(0, 2, 1)
    bias[:, 0, 15] = inputs["target_fc5b"][sl, 0]
    bias = np.ascontiguousarray(bias.transpose(1, 0, 2).reshape(128, -1))

    return {"wslab": wslab, "ximg": ximg, "w5img": w5img, "bias": bias}


def kernel(**inputs):
    inputs = {k: np.asarray(v) for k, v in inputs.items()}
    nc = _get_nc()
    in_maps = [_prep_core(inputs, c) for c in range(N_CORES)]
    res = run_bass_kernel_spmd(nc, in_maps, list(range(N_CORES)))
    out = np.concatenate([np.asarray(res.results[c]["out"]) for c in range(N_CORES)], axis=0)
    return out.reshape(B, 8, 8).astype(np.float32)


# revision 20
# speedup vs baseline: 1.2176x; 1.0353x over previous
"""Trainium2 Bass kernel for nn_BaselineTargetHead (per-sample dynamic MLP).

Strategy: data-parallel over 8 NeuronCores, 8 samples per core.
Per sample the chain is 5 per-sample linear layers over 64 spatial positions:
  [1024,2048] @ [2048,64] -> sigmoid -> ... -> [1,128] @ [128,64] + b

fc1-fc4 weights (99.9% of bytes) and the input x ship as fp8 e3m4 (4
mantissa bits). Host pre-scales weights by 64 (x by 2) to center N(0,0.02)
data in e3m4's normal range; the inverse scale folds into the ScalarE
activation's `scale`. fc5 weights stay fp16: the output is a 128-term dot
product with no downstream averaging, so fc5 quantization dominates the
error budget (quantizing w5 alone costs 1.4e-2 rel err; w1-w4 cost ~1e-3).

The kernel sits at the ridge: Tensor ~61 us busy (45 ns per
LDWEIGHTS+MATMUL pair, 171 pairs/sample) vs DMA ~61-68 us on a single
queue (the two HWDGE queues share ~370 B/ns of fabric, so splitting the
stream gains nothing). Scheduling details that matter:
  - everything lives in SBUF simultaneously (~186 KB/partition), so all
    DMAs are issued upfront with no tile rotation or flow-control stalls.
  - weight DMAs are TYPED fp16 and bitcast to fp8 at the matmul: the DMA
    engine moves ~10% faster with 2-byte elements (368 vs 334 B/ns
    measured on identical shapes/bytes).
  - fc1 is laid out m-major (col = m*2048 + k*128) and shipped in two
    chunks, so each chunk enables complete m-tiles immediately — the last
    sample's fc1 compute overlaps its own DMA tail.
  - the previous sample's tiny fc4/fc5 are interleaved into fc1's m-groups
    so their input activations (314 ns ScalarE latency each) resolve
    behind ~2.7 us of fc1 matmuls instead of stalling the PE (~1.4
    us/sample of layer-boundary gaps otherwise).
  - matmul: lhsT = W^T tile [128(Cin), 128(Cout)] fp8 (FWL halves the
    weight-load time), rhs = activation tile [128(Cin), 64(spatial)] fp16,
    accumulated over Cin tiles in PSUM fp32. ScalarE applies
    scale+bias+sigmoid fused, writing fp16 tiles that feed the next layer
    without any transposition.
"""

import numpy as np
import ml_dtypes

import concourse.bass as bass
import concourse.mybir as mybir
import concourse.tile as tile
from concourse.bass_utils import run_bass_kernel_spmd

N_CORES = 8
B = 64
S_PER_CORE = B // N_CORES  # 8 samples per core
HW = 64  # 8x8 spatial positions
LAYERS = [(2048, 1024), (1024, 512), (512, 256), (256, 128)]  # (Cin, Cout) of fc1..fc4
W_SCALE_FP8 = 64.0  # host multiplies fp8 weights by this; kernel divides back
X_SCALE_FP8 = 2.0  # same for the input x image
A_COLS = (LAYERS[0][0] // 128) * LAYERS[0][1]  # 16384 fp8 cols (fc1, m-major)
B_COLS = sum((ci // 128) * co for ci, co in LAYERS[1:])  # 5376 fp8 cols (fc2-4)
X_COLS = (2048 // 128) * HW  # 1024
W5_COLS = 32  # w5 zero-padded to 32 cols for a legal M=32 matmul
# bias image columns per sample: fc1 m0..7 | fc2 m0..3 | fc3 m0..1 | fc4 m0 | fc5
BIAS_COL0 = [0, 8, 12, 14]
BIAS_COLS = 16
# per-layer PSUM scale to undo the host-side fp8 pre-scaling
ACT_SCALE = [
    1.0 / (W_SCALE_FP8 * X_SCALE_FP8),
    1.0 / W_SCALE_FP8,
    1.0 / W_SCALE_FP8,
    1.0 / W_SCALE_FP8,
]
# per-layer base fp8 column of each layer's weights within a sample's B image
B_OFF = [0, 4096, 4096 + 1024]  # fc2, fc3, fc4


def _split_ctrl_multiwaits(nc):
    """walrus in this env rejects >1 sync-wait per instruction. Move extra
    waits onto NOPs placed immediately before, on the same engine — engines
    execute in order, so this is semantically identical."""
    n_fixed = 0
    for bb in nc.main_func.blocks:
        insts = bb.instructions
        i = 0
        while i < len(insts):
            ins = insts[i]
            si = ins.sync_info
            if si is not None and si.on_wait and len(si.on_wait) > 1:
                waits = list(si.on_wait)
                new_nops = []
                for j, w in enumerate(waits[1:]):
                    nop = mybir.InstNoOp(name=f"{ins.name}-splitw-{j}", ins=[], outs=[])
                    nop.engine = ins.engine
                    nop.sync_info = mybir.SyncInfo(on_update=[], on_wait=[w])
                    new_nops.append(nop)
                si.on_wait = [waits[0]]
                insts[i:i] = new_nops
                i += len(new_nops)
                n_fixed += 1
            i += 1
    return n_fixed


def _build_nc():
    f8 = mybir.dt.float8e3
    f16 = mybir.dt.float16
    f32 = mybir.dt.float32
    nc = bass.Bass()
    # weight/x images carry fp8 bytes but are typed fp16 for the DMA (2-byte
    # elements stream ~10% faster); compute slices bitcast back to fp8.
    wslab_d = nc.dram_tensor(
        "wslab", [S_PER_CORE, 128, (A_COLS + B_COLS) // 2], f16, kind="ExternalInput"
    )
    ximg_d = nc.dram_tensor(
        "ximg", [128, S_PER_CORE * X_COLS // 2], f16, kind="ExternalInput"
    )
    w5img_d = nc.dram_tensor("w5img", [128, S_PER_CORE * W5_COLS], f16, kind="ExternalInput")
    bias_d = nc.dram_tensor("bias", [128, S_PER_CORE * BIAS_COLS], f32, kind="ExternalInput")
    out_d = nc.dram_tensor("out", [1, S_PER_CORE * HW], f32, kind="ExternalOutput")

    sig = mybir.ActivationFunctionType.Sigmoid
    ident = mybir.ActivationFunctionType.Identity

    with tile.TileContext(nc) as tc:
        with (
            tc.tile_pool(name="wpool", bufs=1) as wpool,
            tc.tile_pool(name="qpool", bufs=2) as qpool,
            tc.tile_pool(name="psum", bufs=6, space="PSUM") as psum_pool,
        ):
            # ---- all DMAs issued upfront; everything fits in SBUF ----
            # SP queue: sample 0's x slice first (it gates the first matmul),
            # then the weight slabs, sample-major: fc1 m0-3 | fc1 m4-7 | fc2-4
            # (sample 0's fc1 in four chunks so the PE starts sooner).
            x_sb = wpool.tile([128, S_PER_CORE * X_COLS // 2], f16)
            nc.sync.dma_start(x_sb[:, 0 : X_COLS // 2], ximg_d[:, 0 : X_COLS // 2])
            wa_sb, wb_sb = [], []
            for s in range(S_PER_CORE):
                ta = wpool.tile([128, A_COLS // 2], f16, name=f"wa{s}")
                nchunk = 4 if s == 0 else 2
                step = A_COLS // 2 // nchunk
                for j in range(nchunk):
                    nc.sync.dma_start(
                        ta[:, j * step : (j + 1) * step],
                        wslab_d[s, :, j * step : (j + 1) * step],
                    )
                wa_sb.append(ta)
                tb = wpool.tile([128, B_COLS // 2], f16, name=f"wb{s}")
                nc.sync.dma_start(
                    tb[:], wslab_d[s, :, A_COLS // 2 : (A_COLS + B_COLS) // 2]
                )
                wb_sb.append(tb)
            # ACT queue: rest of x, bias, w5 — all needed only after ~15 us,
            # and this queue drains early so per-sample output DMAs are
            # never stuck behind weight traffic.
            nc.scalar.dma_start(x_sb[:, X_COLS // 2 :], ximg_d[:, X_COLS // 2 :])
            bias_sb = wpool.tile([128, S_PER_CORE * BIAS_COLS], f32)
            nc.scalar.dma_start(bias_sb[:], bias_d[:])
            w5_sb = wpool.tile([128, S_PER_CORE * W5_COLS], f16)
            nc.scalar.dma_start(w5_sb[:], w5img_d[:])
            # all samples' outputs land in partition 0 of one tile
            # (sample s -> columns s*HW..(s+1)*HW) so one DMA ships them all
            ot_all = wpool.tile([128, S_PER_CORE * HW], f32)

            # ---- compute ----
            def w_slice(s, li, k, m):
                """fp8 lhsT [128, 128] for (layer, k-tile, m-tile) of sample s."""
                if li == 0:
                    col = m * 2048 + k * 128  # m-major fc1 layout
                    return wa_sb[s][:, col // 2 : col // 2 + 64].bitcast(
                        mybir.dt.float8e3
                    )
                col = B_OFF[li - 1] + k * LAYERS[li][1] + m * 128
                return wb_sb[s][:, col // 2 : col // 2 + 64].bitcast(mybir.dt.float8e3)

            def x_slice(s, k):
                c = s * X_COLS + k * HW
                return x_sb[:, c // 2 : c // 2 + HW // 2].bitcast(mybir.dt.float8e3)

            q_tiles = [None] * S_PER_CORE  # per-sample [q1, q2, q3, q4]

            def emit_layer(s, li, m_range, q_prev_fn):
                cin, cout = LAYERS[li]
                kt = cin // 128
                qn = q_tiles[s][li]
                for m in m_range:
                    ps = psum_pool.tile([128, HW], f32, tag="ps")
                    for k in range(kt):
                        nc.tensor.matmul(
                            ps[:],
                            w_slice(s, li, k, m),
                            q_prev_fn(k),
                            start=(k == 0),
                            stop=(k == kt - 1),
                        )
                    bcol = s * BIAS_COLS + BIAS_COL0[li] + m
                    nc.scalar.activation(
                        qn[:, m * HW : (m + 1) * HW],
                        ps[:],
                        sig,
                        bias=bias_sb[:, bcol : bcol + 1],
                        scale=ACT_SCALE[li],
                    )

            def emit_fc5(s):
                ps5 = psum_pool.tile([128, HW], f32, tag="ps", name=f"ps5_{s}")
                w5t = w5_sb[:, s * W5_COLS : (s + 1) * W5_COLS]
                nc.tensor.matmul(
                    ps5[0:32, :], w5t, q_tiles[s][3][:, 0:HW], start=True, stop=True
                )
                b5col = s * BIAS_COLS + 15
                nc.scalar.activation(
                    ot_all[0:1, s * HW : (s + 1) * HW],
                    ps5[0:1, :],
                    ident,
                    bias=bias_sb[0:1, b5col : b5col + 1],
                    scale=1.0,
                )
                nc.scalar.dma_start(
                    out_d[0:1, s * HW : (s + 1) * HW],
                    ot_all[0:1, s * HW : (s + 1) * HW],
                )

            for s in range(S_PER_CORE):
                q_tiles[s] = [
                    qpool.tile(
                        [128, (LAYERS[li][1] // 128) * HW],
                        f16,
                        tag=f"q{li}",
                        name=f"q{li}_{s}",
                    )
                    for li in range(4)
                ]
                xf = lambda k, s=s: x_slice(s, k)
                # fc1 m0-3; the previous sample's fc4 resolves its fc3
                # activations behind these 2.7 us of matmuls
                emit_layer(s, 0, range(0, 4), xf)
                if s > 0:
                    emit_layer(s - 1, 3, range(0, 1), lambda k, p=s - 1: q_tiles[p][2][:, k * HW : (k + 1) * HW])
                emit_layer(s, 0, range(4, 8), xf)
                if s > 0:
                    emit_fc5(s - 1)
                emit_layer(s, 1, range(0, 4), lambda k, s=s: q_tiles[s][0][:, k * HW : (k + 1) * HW])
                emit_layer(s, 2, range(0, 2), lambda k, s=s: q_tiles[s][1][:, k * HW : (k + 1) * HW])
            last = S_PER_CORE - 1
            emit_layer(last, 3, range(0, 1), lambda k: q_tiles[last][2][:, k * HW : (k + 1) * HW])
            emit_fc5(last)

    _split_ctrl_multiwaits(nc)
    return nc


_NC_CACHE = None


def _get_nc():
    global _NC_CACHE
    if _NC_CACHE is None:
        _NC_CACHE = _build_nc()
    return _NC_CACHE


def _to_e3m4(a, scale):
    return np.clip(a * scale, -14.0, 14.0).astype(ml_dtypes.float8_e3m4)


def _prep_core(inputs, c):
    """Build the per-core input map (numpy only, host-side layout prep)."""
    sl = slice(c * S_PER_CORE, (c + 1) * S_PER_CORE)

    def wimg(li):
        cin, cout = LAYERS[li]
        w = inputs[f"target_fc{li + 1}w"][sl, :, :, 0, 0]  # [S, Cout, Cin]
        # -> [S, 128, (Cin/128)*Cout] with img[s, p, k*Cout+co] = w[s, co, k*128+p]
        wt = w.transpose(0, 2, 1).reshape(S_PER_CORE, cin // 128, 128, cout)
        return wt.transpose(0, 2, 1, 3).reshape(S_PER_CORE, 128, -1)

    w1 = wimg(0)  # [S, 128, 16384] k-major: col = k*1024 + m*128
    # fc1 -> m-major: col = m*2048 + k*128
    w1 = (
        w1.reshape(S_PER_CORE, 128, 16, 8, 128)
        .transpose(0, 1, 3, 2, 4)
        .reshape(S_PER_CORE, 128, A_COLS)
    )
    wslab = np.ascontiguousarray(
        _to_e3m4(np.concatenate([w1] + [wimg(li) for li in (1, 2, 3)], axis=2), W_SCALE_FP8)
    ).view(np.uint8).reshape(S_PER_CORE, 128, -1).view(np.float16)

    x = inputs["target_in_vec"][sl].reshape(S_PER_CORE, 2048 // 128, 128, HW)
    ximg = x.transpose(2, 0, 1, 3).reshape(128, S_PER_CORE * X_COLS)
    ximg = np.ascontiguousarray(_to_e3m4(ximg, X_SCALE_FP8)).view(np.uint8).view(np.float16)

    w5 = inputs["target_fc5w"][sl, 0, :, 0, 0].astype(np.float16)  # [S, 128]
    w5img = np.zeros((128, S_PER_CORE, W5_COLS), np.float16)
    w5img[:, :, 0] = w5.T
    w5img = np.ascontiguousarray(w5img.reshape(128, -1))

    bias = np.zeros((S_PER_CORE, 128, BIAS_COLS), np.float32)
    for li, (cin, cout) in enumerate(LAYERS):
        b = inputs[f"target_fc{li + 1}b"][sl]  # [S, Cout]
        bias[:, :, BIAS_COL0[li] : BIAS_COL0[li] + cout // 128] = b.reshape(
            S_PER_CORE, cout // 128, 128
        ).transpose(0, 2, 1)
    bias[:, 0, 15] = inputs["target_fc5b"][sl, 0]
    bias = np.ascontiguousarray(bias.transpose(1, 0, 2).reshape(128, -1))

    return {"wslab": wslab, "ximg": ximg, "w5img": w5img, "bias": bias}


def kernel(**inputs):
    inputs = {k: np.asarray(v) for k, v in inputs.items()}
    nc = _get_nc()
    in_maps = [_prep_core(inputs, c) for c in range(N_CORES)]
    res = run_bass_kernel_spmd(nc, in_maps, list(range(N_CORES)))
    out = np.concatenate([np.asarray(res.results[c]["out"]) for c in range(N_CORES)], axis=0)
    return out.reshape(B, 8, 8).astype(np.float32)
